# revision 2
# baseline (speedup 1.0000x reference)
"""Multi-head attention (B=2, N=4096, D=512, H=8) on 8 trn2 NeuronCores.

Sharding: core c handles batch b = c//4 and head-pair p = c%4 (heads 2p,
2p+1).  Each core projects its batch's Q/K/V against its pair's weight
columns, computes transposed attention scores sT = K_h @ Q_h^T, applies
exp((1/sqrt(dk))*sT) on the ACT engine, multiplies by an augmented
V (extra ones column) so the softmax denominators fall out of the same
matmul, and applies its rows of Wo.  Normalization by the softmax
denominator commutes with the output projection (it is a per-query row
scale), so it is applied on the host during the cross-core reduction.

v2: the whole attention + out-projection phase runs in the PE array's
64x128 row-tiled mode (two independent 64-row tiles), processing one
head at a time with that head's q/k duplicated onto both partition
halves.  Scores for even key-chunks run on tile (0,0), odd chunks on
tile (64,0), concurrently; the PV matmul splits each 128-key chunk's
contraction across the two tiles into two PSUM accumulators that are
summed on the DVE.  This keeps the PE in one tiling mode for the whole
phase (no drains) and doubles score throughput.  The ACT engine does
nothing but exp.

Device layouts (host pre-arranges):
  xt{q,k,v}: X^T            [512, N]  (D on partitions when tiled)
  w{q,k,v}:  [128, 4, 128]  w[p, dc, c] = W[dc*128+p, off+c]
  b{q,k,v}:  [128, 1]       pair slice of bias
  wo:        [128, 4, 128]  wo[p, mt, c] = Wo[off+p, mt*128+c]
Outputs per core:
  y0, y1: [512, N]  y_h[dout, q] = (O_un_h @ Wo_h)^T  (unnormalized)
  den:    [2, N]    softmax denominators per head
Final host step: out[b] = (sum_{p,h} y_h / den_h).T + bo
"""

import numpy as np

_B, _N, _D, _H, _DK = 2, 4096, 512, 8, 64
_NCORES = 8

_nc_cache = {}


def _build(n=_N, zero_bias=False):
    import concourse.mybir as mybir
    import concourse.tile as tile
    from concourse import bacc

    f32 = mybir.dt.float32
    bf16 = mybir.dt.float16
    Exp = mybir.ActivationFunctionType.Exp
    D = _D
    NKC = n // 128  # key chunks of 128
    NQC = n // 512  # query chunks of 512
    blocks = []
    i = 0
    while i < NKC:
        blen = min(3, NKC - i)
        blocks.append((i, blen))
        i += blen

    nc = bacc.Bacc(
        "TRN2", target_bir_lowering=False, debug=False, num_devices=_NCORES
    )

    xt = {
        t: nc.dram_tensor(f"xt{t}", [D, n], bf16, kind="ExternalInput").ap()
        for t in "qkv"
    }
    w = {
        t: nc.dram_tensor(f"w{t}", [128, 4, 128], bf16, kind="ExternalInput").ap()
        for t in "qkv"
    }
    bvec = {
        t: nc.dram_tensor(f"b{t}", [128, 1], f32, kind="ExternalInput").ap()
        for t in "qkv"
    }
    wo = nc.dram_tensor("wo", [128, 4, 128], bf16, kind="ExternalInput").ap()
    y_out = [
        nc.dram_tensor(f"y{h}", [D, n], bf16, kind="ExternalOutput").ap()
        for h in range(2)
    ]
    den_out = nc.dram_tensor("den", [2, n], f32, kind="ExternalOutput").ap()

    with tile.TileContext(nc) as tc:
        with (
            tc.tile_pool(name="consts", bufs=1) as consts,
            tc.tile_pool(name="xtp", bufs=8) as xtp,
            tc.tile_pool(name="persist", bufs=1) as persist,
            tc.tile_pool(name="ep", bufs=4) as ep,
            tc.tile_pool(name="otp", bufs=2) as otp,
            tc.tile_pool(name="ysb", bufs=4) as ysbp,
            tc.tile_pool(name="psA", bufs=2, space="PSUM") as psA,
            tc.tile_pool(name="psB", bufs=2, space="PSUM") as psB,
        ):
            from concourse.masks import make_identity

            ident = consts.tile([128, 128], bf16, name="ident")
            make_identity(nc, ident)
            wsb, bsb = {}, {}
            for t in "qkv":
                wsb[t] = consts.tile([128, 4, 128], bf16, name=f"w{t}sb", tag=f"w{t}sb")
                nc.sync.dma_start(out=wsb[t], in_=w[t])
                bsb[t] = consts.tile([128, 1], f32, name=f"b{t}sb", tag=f"b{t}sb")
                nc.sync.dma_start(out=bsb[t], in_=bvec[t])
            wosb = consts.tile([128, 4, 128], bf16, name="wosb", tag="wosb")
            nc.sync.dma_start(out=wosb, in_=wo)

            # Persistent SBUF tensors.
            # qtd/ktd[h][nk]: head h's 64 dims duplicated on both halves.
            qtd = [
                [
                    persist.tile([128, 512], bf16, name=f"qtd{h}_{i}", tag=f"qtd{h}_{i}")
                    for i in range(NQC)
                ]
                for h in range(2)
            ]
            ktd = [
                [
                    persist.tile([128, 512], bf16, name=f"ktd{h}_{i}", tag=f"ktd{h}_{i}")
                    for i in range(NQC)
                ]
                for h in range(2)
            ]
            vt_t = [
                persist.tile([128, 512], bf16, name=f"vt{i}", tag=f"vt{i}")
                for i in range(NQC)
            ]
            # vch0[c]: [128 keys, 65]: cols 0-63 = V_h0 dims, col 64 = ones
            # vch1[c]: [128 keys, 128]: cols 0-62 = 0, col 63 = ones,
            #          cols 64-127 = V_h1 dims
            vch0 = [
                persist.tile([128, 65], bf16, name=f"vch0_{c}", tag=f"vch0_{c}")
                for c in range(NKC)
            ]
            vch1 = [
                persist.tile([128, 128], bf16, name=f"vch1_{c}", tag=f"vch1_{c}")
                for c in range(NKC)
            ]
            den_sb = persist.tile([66, n], f32, name="den", tag="den")
            for c in range(NKC):
                nc.vector.memset(vch0[c][:, 64:65], 1.0)
                nc.vector.memset(vch1[c][:, 0:63], 0.0)
                nc.vector.memset(vch1[c][:, 63:64], 1.0)

            # ---- phase 1: projections (full 128x128 mode) ----
            for nk in range(NQC):
                for t in "qkv":
                    ppsum = psA.tile([128, 512], f32, name=f"pp_{t}{nk}", tag="s")
                    for dc in range(4):
                        xtile = xtp.tile(
                            [128, 512], bf16, name=f"x_{t}{nk}{dc}", tag="xt"
                        )
                        nc.sync.dma_start(
                            out=xtile,
                            in_=xt[t][dc * 128 : (dc + 1) * 128, nk * 512 : (nk + 1) * 512],
                        )
                        nc.tensor.matmul(
                            ppsum,
                            wsb[t][:, dc, :],
                            xtile,
                            start=(dc == 0),
                            stop=(dc == 3),
                        )
                    if t == "v":
                        if zero_bias:
                            nc.vector.tensor_copy(out=vt_t[nk], in_=ppsum)
                        else:
                            nc.vector.tensor_scalar_add(
                                out=vt_t[nk], in0=ppsum, scalar1=bsb[t]
                            )
                    else:
                        dst = qtd if t == "q" else ktd
                        for h in range(2):
                            half = slice(h * 64, (h + 1) * 64)
                            other = slice(64, 128) if h == 0 else slice(0, 64)
                            if zero_bias:
                                nc.vector.tensor_copy(
                                    out=dst[h][nk][half, :], in_=ppsum[half, :]
                                )
                            else:
                                nc.vector.tensor_scalar_add(
                                    out=dst[h][nk][half, :],
                                    in0=ppsum[half, :],
                                    scalar1=bsb[t][half, :],
                                )
                            # duplicate onto the other partition half
                            nc.sync.dma_start(
                                out=dst[h][nk][other, :], in_=dst[h][nk][half, :]
                            )
                # v^T chunks -> per-head augmented layouts
                for c in range(nk * 4, nk * 4 + 4):
                    pt = psA.tile([128, 512], bf16, name=f"pt{c}", tag="s")
                    nc.tensor.transpose(
                        pt[:, 0:128],
                        vt_t[c // 4][:, (c % 4) * 128 : (c % 4 + 1) * 128],
                        ident,
                    )
                    nc.vector.tensor_copy(out=vch0[c][:, 0:64], in_=pt[:, 0:64])
                    nc.vector.tensor_copy(out=vch1[c][:, 64:128], in_=pt[:, 64:128])

            # ---- phase 2+3: attention + out-projection, all in 64x128
            # row-tiled mode, one head at a time ----
            for qc in range(NQC):
                qs = slice(qc * 512, (qc + 1) * 512)
                ot_stack = otp.tile([128, 512], bf16, name=f"ot{qc}", tag="ot")
                for h in range(2):
                    o_lo = psB.tile([128, 512], f32, name=f"olo_{h}_{qc}", tag="oy")
                    o_hi = psB.tile([128, 512], f32, name=f"ohi_{h}_{qc}", tag="oy")
                    vch = vch0 if h == 0 else vch1
                    ow = 65 if h == 0 else 128
                    for k0, blen in blocks:
                        s_ps = psA.tile(
                            [128, blen * 512], f32, name=f"s_{h}_{qc}_{k0}", tag="s"
                        )
                        for j in range(blen):
                            kc = k0 + j
                            hp = slice(0, 64) if kc % 2 == 0 else slice(64, 128)
                            nc.tensor.matmul(
                                s_ps[:, j * 512 : (j + 1) * 512],
                                ktd[h][kc // 4][hp, (kc % 4) * 128 : (kc % 4 + 1) * 128],
                                qtd[h][qc][hp, :],
                                start=True,
                                stop=True,
                                skip_group_check=True,
                            )
                        e_sb = ep.tile(
                            [128, blen * 512], bf16, name=f"e_{h}_{qc}_{k0}", tag="e"
                        )
                        nc.scalar.activation(e_sb, s_ps, Exp, scale=0.125)
                        for j in range(blen):
                            kc = k0 + j
                            st, sp = (kc == 0), (kc == NKC - 1)
                            es = e_sb[:, j * 512 : (j + 1) * 512]
                            nc.tensor.matmul(
                                o_lo[0:ow, :],
                                vch[kc][0:64, :],
                                es[0:64, :],
                                start=st,
                                stop=sp,
                                skip_group_check=True,
                            )
                            nc.tensor.matmul(
                                o_hi[0:ow, :],
                                vch[kc][64:128, :],
                                es[64:128, :],
                                start=st,
                                stop=sp,
                                skip_group_check=True,
                            )
                    if h == 0:
                        nc.vector.tensor_add(
                            out=ot_stack[0:64, :], in0=o_lo[0:64, :], in1=o_hi[0:64, :]
                        )
                        nc.vector.tensor_add(
                            out=den_sb[64:65, qs],
                            in0=o_lo[64:65, :],
                            in1=o_hi[64:65, :],
                        )
                    else:
                        nc.vector.tensor_add(
                            out=ot_stack[64:128, :],
                            in0=o_lo[64:128, :],
                            in1=o_hi[64:128, :],
                        )
                        nc.vector.tensor_add(
                            out=den_sb[63:64, qs],
                            in0=o_lo[63:64, :],
                            in1=o_hi[63:64, :],
                        )

                # out-projection for this q chunk (same tiled mode)
                for mt in range(4):
                    for h in range(2):
                        hp = slice(h * 64, (h + 1) * 64)
                        y_ps = psB.tile(
                            [128, 512], f32, name=f"y_{h}_{qc}_{mt}", tag="oy"
                        )
                        nc.tensor.matmul(
                            y_ps,
                            wosb[hp, mt, :],
                            ot_stack[hp, :],
                            start=True,
                            stop=True,
                            skip_group_check=True,
                        )
                        y_sb = ysbp.tile(
                            [128, 512], bf16, name=f"ysb_{h}_{qc}_{mt}", tag="ysb"
                        )
                        nc.vector.tensor_copy(out=y_sb, in_=y_ps)
                        nc.sync.dma_start(
                            out=y_out[h][mt * 128 : (mt + 1) * 128, qs], in_=y_sb
                        )
            nc.sync.dma_start(out=den_out[0:1, :], in_=den_sb[64:65, :])
            nc.sync.dma_start(out=den_out[1:2, :], in_=den_sb[63:64, :])
    nc.finalize()
    return nc


def get_nc(n=_N, zero_bias=False):
    key = (n, zero_bias)
    if key not in _nc_cache:
        _nc_cache[key] = _build(n, zero_bias)
    return _nc_cache[key]


def make_in_maps(Q, K, V, Wq, bq, Wk, bk, Wv, bv, Wo, bo, n=_N):
    """Per-core input dicts (host-side sharding / layout prep)."""
    bf = np.float16
    xts = {}
    for b in range(_B):
        xts[b] = {
            "xtq": np.ascontiguousarray(Q[b][:n].T.astype(bf)),
            "xtk": np.ascontiguousarray(K[b][:n].T.astype(bf)),
            "xtv": np.ascontiguousarray(V[b][:n].T.astype(bf)),
        }
    in_maps = []
    for c in range(_NCORES):
        b, p = divmod(c, 4)
        off = p * 128
        m = dict(xts[b])
        for t, W, bias in (("q", Wq, bq), ("k", Wk, bk), ("v", Wv, bv)):
            m[f"w{t}"] = np.ascontiguousarray(
                W[:, off : off + 128].reshape(4, 128, 128).transpose(1, 0, 2).astype(bf)
            )
            m[f"b{t}"] = np.ascontiguousarray(bias[off : off + 128].reshape(128, 1))
        m["wo"] = np.ascontiguousarray(
            Wo[off : off + 128].reshape(128, 4, 128).astype(bf)
        )
        in_maps.append(m)
    return in_maps


def assemble(results, bo, n=_N):
    """Cross-core reduction: normalize by softmax denominators, sum heads,
    add output bias, restore [B, N, D] layout."""
    out = np.empty((_B, n, _D), np.float32)
    for b in range(_B):
        acc = np.zeros((_D, n), np.float32)
        for p in range(4):
            r = results[4 * b + p]
            for h in range(2):
                acc += r[f"y{h}"].astype(np.float32) / r["den"][h][None, :]
        out[b] = acc.T + bo
    return out


def kernel(Q, K, V, Wq, bq, Wk, bk, Wv, bv, Wo, bo):
    from concourse import bass_utils

    args = [np.asarray(a, np.float32) for a in (Q, K, V, Wq, bq, Wk, bk, Wv, bv, Wo, bo)]
    Q, K, V, Wq, bq, Wk, bk, Wv, bv, Wo, bo = args
    zb = not (np.any(bq) or np.any(bk) or np.any(bv))
    nc = get_nc(zero_bias=zb)
    in_maps = make_in_maps(Q, K, V, Wq, bq, Wk, bk, Wv, bv, Wo, bo)
    res = bass_utils.run_bass_kernel_spmd(
        nc, in_maps, core_ids=list(range(_NCORES))
    )
    return assemble(res.results, bo)


# revision 12
# speedup vs baseline: 1.1278x; 1.1278x over previous
"""Multi-head attention (B=2, N=4096, D=512, H=8) on 8 trn2 NeuronCores.

Sharding: core c handles batch b = c//4 and head-pair p = c%4 (heads 2p,
2p+1).  Each core projects its batch's Q/K/V against its pair's weight
columns, computes transposed attention scores sT = K_h @ Q_h^T, applies
exp((1/sqrt(dk))*sT) on the ACT engine, multiplies by an augmented
V (extra ones column) so the softmax denominators fall out of the same
matmul, and applies its rows of Wo.  Normalization by the softmax
denominator commutes with the output projection (it is a per-query row
scale), so it is applied on the host during the cross-core reduction.

v2: the whole attention + out-projection phase runs in the PE array's
64x128 row-tiled mode (two independent 64-row tiles), processing one
head at a time with that head's q/k duplicated onto both partition
halves.  Scores for even key-chunks run on tile (0,0), odd chunks on
tile (64,0), concurrently; the PV matmul splits each 128-key chunk's
contraction across the two tiles into two PSUM accumulators that are
summed on the DVE.  This keeps the PE in one tiling mode for the whole
phase (no drains) and doubles score throughput.  The ACT engine does
nothing but exp.

Device layouts (host pre-arranges):
  xt{q,k,v}: X^T            [512, N]  (D on partitions when tiled)
  w{q,k,v}:  [128, 4, 128]  w[p, dc, c] = W[dc*128+p, off+c]
  b{q,k,v}:  [128, 1]       pair slice of bias
  wo:        [128, 4, 128]  wo[p, mt, c] = Wo[off+p, mt*128+c]
Outputs per core:
  y0, y1: [512, N]  y_h[dout, q] = (O_un_h @ Wo_h)^T  (unnormalized)
  den:    [2, N]    softmax denominators per head
Final host step: out[b] = (sum_{p,h} y_h / den_h).T + bo
"""

import numpy as np

_B, _N, _D, _H, _DK = 2, 4096, 512, 8, 64
_NCORES = 8

_nc_cache = {}


def _enable_ldw_opt():
    """Let the backend overlap PE weight loads with matmuls (the default
    compile flags disable it)."""
    try:
        from concourse import compiler_utils

        flags = compiler_utils.get_compiler_flags()
        patched = [f.replace("--enable-ldw-opt=false", "--enable-ldw-opt=true") for f in flags]
        if patched != flags:
            compiler_utils.set_compiler_flags(patched)
    except Exception:
        pass


def _build(n=_N, zero_bias=False):
    import concourse.mybir as mybir
    import concourse.tile as tile
    from concourse import bacc

    f32 = mybir.dt.float32
    bf16 = mybir.dt.float16
    Exp = mybir.ActivationFunctionType.Exp
    D = _D
    NKC = n // 128  # key chunks of 128
    NQC = n // 512  # query chunks of 512
    blocks = []
    i = 0
    while i < NKC:
        blen = min(3, NKC - i)
        blocks.append((i, blen))
        i += blen

    nc = bacc.Bacc(
        "TRN2", target_bir_lowering=False, debug=False, num_devices=_NCORES
    )

    # inputs pre-tiled on host so every DMA is a contiguous 256KB block
    # with 2KB per partition line: xt[t][dc, nk2, p, c] =
    # X^T[dc*128+p, nk2*1024+c]
    xt = {
        t: nc.dram_tensor(
            f"xt{t}", [4, n // 1024, 128, 1024], bf16, kind="ExternalInput"
        ).ap()
        for t in "qkv"
    }
    w = {
        t: nc.dram_tensor(f"w{t}", [128, 4, 128], bf16, kind="ExternalInput").ap()
        for t in "qkv"
    }
    bvec = {
        t: nc.dram_tensor(f"b{t}", [128, 1], f32, kind="ExternalInput").ap()
        for t in "qkv"
    }
    wo = nc.dram_tensor("wo", [128, 4, 128], bf16, kind="ExternalInput").ap()
    # outputs tiled likewise: y[h][mt, nk, p, c] = y_h[mt*128+p, nk*512+c]
    y_out = [
        nc.dram_tensor(
            f"y{h}", [4, n // 512, 128, 512], bf16, kind="ExternalOutput"
        ).ap()
        for h in range(2)
    ]
    den_out = nc.dram_tensor("den", [2, n], f32, kind="ExternalOutput").ap()

    with tile.TileContext(nc) as tc:
        with (
            tc.tile_pool(name="consts", bufs=1) as consts,
            tc.tile_pool(name="xtp", bufs=8) as xtp,
            tc.tile_pool(name="persist", bufs=1) as persist,
            tc.tile_pool(name="ep", bufs=4) as ep,
            tc.tile_pool(name="otp", bufs=2) as otp,
            tc.tile_pool(name="ysb", bufs=4) as ysbp,
            tc.tile_pool(name="ohp", bufs=2) as ohp,
            tc.tile_pool(name="psA", bufs=2, space="PSUM") as psA,
            tc.tile_pool(name="psB", bufs=2, space="PSUM") as psB,
        ):
            from concourse.masks import make_identity

            ident = consts.tile([128, 128], bf16, name="ident")
            make_identity(nc, ident)
            wsb, bsb = {}, {}
            for t in "qkv":
                wsb[t] = consts.tile([128, 4, 128], bf16, name=f"w{t}sb", tag=f"w{t}sb")
                nc.sync.dma_start(out=wsb[t], in_=w[t])
                bsb[t] = consts.tile([128, 1], f32, name=f"b{t}sb", tag=f"b{t}sb")
                nc.sync.dma_start(out=bsb[t], in_=bvec[t])
            wosb = consts.tile([128, 4, 128], bf16, name="wosb", tag="wosb")
            nc.sync.dma_start(out=wosb, in_=wo)

            # Persistent SBUF tensors.
            # qtd/ktd[h][nk]: head h's 64 dims duplicated on both halves.
            qtd = [
                [
                    persist.tile([128, 512], bf16, name=f"qtd{h}_{i}", tag=f"qtd{h}_{i}")
                    for i in range(NQC)
                ]
                for h in range(2)
            ]
            ktd = [
                [
                    persist.tile([128, 512], bf16, name=f"ktd{h}_{i}", tag=f"ktd{h}_{i}")
                    for i in range(NQC)
                ]
                for h in range(2)
            ]
            vt_t = [
                persist.tile([128, 512], bf16, name=f"vt{i}", tag=f"vt{i}")
                for i in range(NQC)
            ]
            # vch0[c]: [128 keys, 65]: cols 0-63 = V_h0 dims, col 64 = ones
            # vch1[c]: [128 keys, 128]: col 0 = ones, cols 1-63 = 0,
            #          cols 64-127 = V_h1 dims
            vch0 = [
                persist.tile([128, 65], bf16, name=f"vch0_{c}", tag=f"vch0_{c}")
                for c in range(NKC)
            ]
            vch1 = [
                persist.tile([128, 128], bf16, name=f"vch1_{c}", tag=f"vch1_{c}")
                for c in range(NKC)
            ]
            den_sb = persist.tile([66, n], f32, name="den", tag="den")
            for c in range(NKC):
                nc.vector.memset(vch0[c][:, 64:65], 1.0)
                nc.vector.memset(vch1[c][:, 0:1], 1.0)
                nc.vector.memset(vch1[c][:, 1:64], 0.0)

            # ---- phase 1: projections (full 128x128 mode) ----
            for nk2 in range(n // 1024):
                for t in "qkv":
                    xtiles = []
                    for dc in range(4):
                        xtile = xtp.tile(
                            [128, 1024], bf16, name=f"x_{t}{nk2}{dc}", tag="xt"
                        )
                        nc.sync.dma_start(out=xtile, in_=xt[t][dc, nk2])
                        xtiles.append(xtile)
                    for half in range(2):
                        nk = nk2 * 2 + half
                        hs = slice(half * 512, (half + 1) * 512)
                        ppsum = psA.tile([128, 512], f32, name=f"pp_{t}{nk}", tag="s")
                        for dc in range(4):
                            nc.tensor.matmul(
                                ppsum,
                                wsb[t][:, dc, :],
                                xtiles[dc][:, hs],
                                start=(dc == 0),
                                stop=(dc == 3),
                            )
                        if t == "v":
                            if zero_bias:
                                nc.vector.tensor_copy(out=vt_t[nk], in_=ppsum)
                            else:
                                nc.vector.tensor_scalar_add(
                                    out=vt_t[nk], in0=ppsum, scalar1=bsb[t]
                                )
                        else:
                            dst = qtd if t == "q" else ktd
                            for h in range(2):
                                hh = slice(h * 64, (h + 1) * 64)
                                other = slice(64, 128) if h == 0 else slice(0, 64)
                                if zero_bias:
                                    nc.vector.tensor_copy(
                                        out=dst[h][nk][hh, :], in_=ppsum[hh, :]
                                    )
                                else:
                                    nc.vector.tensor_scalar_add(
                                        out=dst[h][nk][hh, :],
                                        in0=ppsum[hh, :],
                                        scalar1=bsb[t][hh, :],
                                    )
                                # duplicate onto the other partition half
                                nc.sync.dma_start(
                                    out=dst[h][nk][other, :], in_=dst[h][nk][hh, :]
                                )
                # v^T chunks -> per-head augmented layouts
                for c in range(nk2 * 8, nk2 * 8 + 8):
                    pt = psA.tile([128, 512], bf16, name=f"pt{c}", tag="s")
                    nc.tensor.transpose(
                        pt[:, 0:128],
                        vt_t[c // 4][:, (c % 4) * 128 : (c % 4 + 1) * 128],
                        ident,
                    )
                    nc.vector.tensor_copy(out=vch0[c][:, 0:64], in_=pt[:, 0:64])
                    nc.vector.tensor_copy(out=vch1[c][:, 64:128], in_=pt[:, 64:128])

            # ---- phase 2+3: attention + out-projection, all in 64x128
            # row-tiled mode, one head at a time ----
            for qc in range(NQC):
                qs = slice(qc * 512, (qc + 1) * 512)
                ot_stack = otp.tile([128, 512], bf16, name=f"ot{qc}", tag="ot")
                for h in range(2):
                    o_lo = psB.tile([128, 512], f32, name=f"olo_{h}_{qc}", tag="oy")
                    o_hi = psB.tile([128, 512], f32, name=f"ohi_{h}_{qc}", tag="oy")
                    vch = vch0 if h == 0 else vch1
                    ow = 65 if h == 0 else 128
                    for k0, blen in blocks:
                        s_ps = psA.tile(
                            [128, blen * 512], f32, name=f"s_{h}_{qc}_{k0}", tag="s"
                        )
                        for j in range(blen):
                            kc = k0 + j
                            hp = slice(0, 64) if kc % 2 == 0 else slice(64, 128)
                            nc.tensor.matmul(
                                s_ps[:, j * 512 : (j + 1) * 512],
                                ktd[h][kc // 4][hp, (kc % 4) * 128 : (kc % 4 + 1) * 128],
                                qtd[h][qc][hp, :],
                                start=True,
                                stop=True,
                                skip_group_check=True,
                            )
                        e_sb = ep.tile(
                            [128, blen * 512], bf16, name=f"e_{h}_{qc}_{k0}", tag="e"
                        )
                        nc.scalar.activation(e_sb, s_ps, Exp, scale=0.125)
                        for j in range(blen):
                            kc = k0 + j
                            st, sp = (kc == 0), (kc == NKC - 1)
                            es = e_sb[:, j * 512 : (j + 1) * 512]
                            nc.tensor.matmul(
                                o_lo[0:ow, :],
                                vch[kc][0:64, :],
                                es[0:64, :],
                                start=st,
                                stop=sp,
                                skip_group_check=True,
                            )
                            nc.tensor.matmul(
                                o_hi[0:ow, :],
                                vch[kc][64:128, :],
                                es[64:128, :],
                                start=st,
                                stop=sp,
                                skip_group_check=True,
                            )
                    # DVE may read only one PSUM input: stage o_hi in SBUF
                    o_hi_sb = ohp.tile([128, 512], f32, name=f"ohs_{h}_{qc}", tag="oh")
                    nc.vector.tensor_copy(out=o_hi_sb, in_=o_hi)
                    if h == 0:
                        nc.vector.tensor_add(
                            out=ot_stack[0:64, :],
                            in0=o_lo[0:64, :],
                            in1=o_hi_sb[0:64, :],
                        )
                        nc.vector.tensor_add(
                            out=den_sb[64:65, qs],
                            in0=o_lo[64:65, :],
                            in1=o_hi_sb[64:65, :],
                        )
                    else:
                        nc.vector.tensor_add(
                            out=ot_stack[64:128, :],
                            in0=o_lo[64:128, :],
                            in1=o_hi_sb[64:128, :],
                        )
                        nc.vector.tensor_add(
                            out=den_sb[0:1, qs],
                            in0=o_lo[0:1, :],
                            in1=o_hi_sb[0:1, :],
                        )

                # out-projection for this q chunk (same tiled mode)
                for mt in range(4):
                    for h in range(2):
                        hp = slice(h * 64, (h + 1) * 64)
                        y_ps = psB.tile(
                            [128, 512], f32, name=f"y_{h}_{qc}_{mt}", tag="oy"
                        )
                        nc.tensor.matmul(
                            y_ps,
                            wosb[hp, mt, :],
                            ot_stack[hp, :],
                            start=True,
                            stop=True,
                            skip_group_check=True,
                        )
                        y_sb = ysbp.tile(
                            [128, 512], bf16, name=f"ysb_{h}_{qc}_{mt}", tag="ysb"
                        )
                        nc.vector.tensor_copy(out=y_sb, in_=y_ps)
                        nc.sync.dma_start(out=y_out[h][mt, qc], in_=y_sb)
            nc.sync.dma_start(out=den_out[0:1, :], in_=den_sb[64:65, :])
            nc.sync.dma_start(out=den_out[1:2, :], in_=den_sb[0:1, :])
    nc.finalize()
    return nc


def get_nc(n=_N, zero_bias=False):
    key = (n, zero_bias)
    if key not in _nc_cache:
        _nc_cache[key] = _build(n, zero_bias)
    return _nc_cache[key]


def make_in_maps(Q, K, V, Wq, bq, Wk, bk, Wv, bv, Wo, bo, n=_N):
    """Per-core input dicts (host-side sharding / layout prep)."""
    bf = np.float16

    def tile_xt(X):
        # [n, 512] -> X^T tiled as [4, n//1024, 128, 1024]
        xt = X[:n].T.astype(bf)  # [512, n]
        return np.ascontiguousarray(
            xt.reshape(4, 128, n // 1024, 1024).transpose(0, 2, 1, 3)
        )

    xts = {}
    for b in range(_B):
        xts[b] = {
            "xtq": tile_xt(Q[b]),
            "xtk": tile_xt(K[b]),
            "xtv": tile_xt(V[b]),
        }
    in_maps = []
    for c in range(_NCORES):
        b, p = divmod(c, 4)
        off = p * 128
        m = dict(xts[b])
        for t, W, bias in (("q", Wq, bq), ("k", Wk, bk), ("v", Wv, bv)):
            m[f"w{t}"] = np.ascontiguousarray(
                W[:, off : off + 128].reshape(4, 128, 128).transpose(1, 0, 2).astype(bf)
            )
            m[f"b{t}"] = np.ascontiguousarray(bias[off : off + 128].reshape(128, 1))
        m["wo"] = np.ascontiguousarray(
            Wo[off : off + 128].reshape(128, 4, 128).astype(bf)
        )
        in_maps.append(m)
    return in_maps


def assemble(results, bo, n=_N):
    """Cross-core reduction: normalize by softmax denominators, sum heads,
    add output bias, restore [B, N, D] layout."""
    out = np.empty((_B, n, _D), np.float32)
    for b in range(_B):
        acc = np.zeros((_D, n), np.float32)
        for p in range(4):
            r = results[4 * b + p]
            for h in range(2):
                y = r[f"y{h}"].transpose(0, 2, 1, 3).reshape(_D, n)
                acc += y.astype(np.float32) / r["den"][h][None, :]
        out[b] = acc.T + bo
    return out


def kernel(Q, K, V, Wq, bq, Wk, bk, Wv, bv, Wo, bo):
    from concourse import bass_utils

    args = [np.asarray(a, np.float32) for a in (Q, K, V, Wq, bq, Wk, bk, Wv, bv, Wo, bo)]
    Q, K, V, Wq, bq, Wk, bk, Wv, bv, Wo, bo = args
    zb = not (np.any(bq) or np.any(bk) or np.any(bv))
    nc = get_nc(zero_bias=zb)
    in_maps = make_in_maps(Q, K, V, Wq, bq, Wk, bk, Wv, bv, Wo, bo)
    res = bass_utils.run_bass_kernel_spmd(
        nc, in_maps, core_ids=list(range(_NCORES))
    )
    return assemble(res.results, bo)


# revision 15
# speedup vs baseline: 1.4179x; 1.2573x over previous
"""Multi-head attention (B=2, N=4096, D=512, H=8) on 8 trn2 NeuronCores.

Sharding: core c handles batch b = c//4 and head-pair p = c%4 (heads 2p,
2p+1).  Each core projects its batch's Q/K/V against its pair's weight
columns, computes transposed attention scores sT = K_h @ Q_h^T, applies
exp((1/sqrt(dk))*sT) on the ACT engine, multiplies by an augmented
V (extra ones column) so the softmax denominators fall out of the same
matmul, and applies its rows of Wo.  Normalization by the softmax
denominator commutes with the output projection (it is a per-query row
scale), so it is applied on the host during the cross-core reduction.

v3: the whole attention + out-projection phase runs in the PE array's
64x128 row-tiled mode (two independent 64-row tiles), processing one
head at a time with that head's q/k duplicated onto both partition
halves.  Scores for even key-chunks run on tile (0,0), odd chunks on
tile (64,0), concurrently; the PV matmul splits each 128-key chunk's
contraction across the two tiles into two PSUM accumulators that are
summed on the DVE.  This keeps the PE in one tiling mode for the whole
phase (no drains) and doubles score throughput.  The ACT engine does
nothing but exp.  Emission is software-pipelined: s/exp run one block
ahead of PV; the out-projection of q-chunk qc-1 and the (deferred,
split-contraction) q projections are slotted into the block stream so
neither the PE nor ACT idles at chunk boundaries.  Phase 1 only
projects k, v and the first q pair, shrinking the DMA-bound prologue;
inputs/outputs are host-tiled so every DMA moves contiguous blocks
with 2KB partition lines.

Device layouts (host pre-arranges):
  xt{q,k,v}: X^T            [512, N]  (D on partitions when tiled)
  w{q,k,v}:  [128, 4, 128]  w[p, dc, c] = W[dc*128+p, off+c]
  b{q,k,v}:  [128, 1]       pair slice of bias
  wo:        [128, 4, 128]  wo[p, mt, c] = Wo[off+p, mt*128+c]
Outputs per core:
  y0, y1: [512, N]  y_h[dout, q] = (O_un_h @ Wo_h)^T  (unnormalized)
  den:    [2, N]    softmax denominators per head
Final host step: out[b] = (sum_{p,h} y_h / den_h).T + bo
"""

import numpy as np

_B, _N, _D, _H, _DK = 2, 4096, 512, 8, 64
_NCORES = 8

_nc_cache = {}


def _enable_ldw_opt():
    """Let the backend overlap PE weight loads with matmuls (the default
    compile flags disable it)."""
    try:
        from concourse import compiler_utils

        flags = compiler_utils.get_compiler_flags()
        patched = [f.replace("--enable-ldw-opt=false", "--enable-ldw-opt=true") for f in flags]
        if patched != flags:
            compiler_utils.set_compiler_flags(patched)
    except Exception:
        pass


def _build(n=_N, zero_bias=False):
    import concourse.mybir as mybir
    import concourse.tile as tile
    from concourse import bacc

    f32 = mybir.dt.float32
    bf16 = mybir.dt.float16
    Exp = mybir.ActivationFunctionType.Exp
    D = _D
    NKC = n // 128  # key chunks of 128
    NQC = n // 512  # query chunks of 512
    blocks = []
    i = 0
    while i < NKC:
        blen = min(3, NKC - i)
        blocks.append((i, blen))
        i += blen

    nc = bacc.Bacc(
        "TRN2", target_bir_lowering=False, debug=False, num_devices=_NCORES
    )

    # inputs pre-tiled on host so every DMA is a contiguous 256KB block
    # with 2KB per partition line: xt[t][dc, nk2, p, c] =
    # X^T[dc*128+p, nk2*1024+c]
    xt = {
        t: nc.dram_tensor(
            f"xt{t}", [4, n // 1024, 128, 1024], bf16, kind="ExternalInput"
        ).ap()
        for t in "qkv"
    }
    w = {
        t: nc.dram_tensor(f"w{t}", [128, 4, 128], bf16, kind="ExternalInput").ap()
        for t in "qkv"
    }
    bvec = {
        t: nc.dram_tensor(f"b{t}", [128, 1], f32, kind="ExternalInput").ap()
        for t in "qkv"
    }
    wo = nc.dram_tensor("wo", [128, 4, 128], bf16, kind="ExternalInput").ap()
    # outputs tiled likewise: y[h][mt, nk, p, c] = y_h[mt*128+p, nk*512+c]
    y_out = [
        nc.dram_tensor(
            f"y{h}", [4, n // 512, 128, 512], bf16, kind="ExternalOutput"
        ).ap()
        for h in range(2)
    ]
    den_out = nc.dram_tensor("den", [2, n], f32, kind="ExternalOutput").ap()

    with tile.TileContext(nc) as tc:
        with (
            tc.tile_pool(name="consts", bufs=1) as consts,
            tc.tile_pool(name="xtp", bufs=8) as xtp,
            tc.tile_pool(name="persist", bufs=1) as persist,
            tc.tile_pool(name="ep", bufs=4) as ep,
            tc.tile_pool(name="otp", bufs=2) as otp,
            tc.tile_pool(name="ysb", bufs=4) as ysbp,
            tc.tile_pool(name="ohp", bufs=2) as ohp,
            tc.tile_pool(name="psA", bufs=2, space="PSUM") as psA,
            tc.tile_pool(name="psB", bufs=2, space="PSUM") as psB,
        ):
            from concourse.masks import make_identity

            ident = consts.tile([128, 128], bf16, name="ident")
            make_identity(nc, ident)
            wsb, bsb = {}, {}
            for t in "qkv":
                wsb[t] = consts.tile([128, 4, 128], bf16, name=f"w{t}sb", tag=f"w{t}sb")
                nc.sync.dma_start(out=wsb[t], in_=w[t])
                bsb[t] = consts.tile([128, 1], f32, name=f"b{t}sb", tag=f"b{t}sb")
                nc.sync.dma_start(out=bsb[t], in_=bvec[t])
            wosb = consts.tile([128, 4, 128], bf16, name="wosb", tag="wosb")
            nc.sync.dma_start(out=wosb, in_=wo)

            # Persistent SBUF tensors.
            # qtd/ktd[h][nk]: head h's 64 dims duplicated on both halves.
            qtd = [
                [
                    persist.tile([128, 512], bf16, name=f"qtd{h}_{i}", tag=f"qtd{h}_{i}")
                    for i in range(NQC)
                ]
                for h in range(2)
            ]
            ktd = [
                [
                    persist.tile([128, 512], bf16, name=f"ktd{h}_{i}", tag=f"ktd{h}_{i}")
                    for i in range(NQC)
                ]
                for h in range(2)
            ]
            vt_t = [
                persist.tile([128, 512], bf16, name=f"vt{i}", tag=f"vt{i}")
                for i in range(NQC)
            ]
            # vch0[c]: [128 keys, 65]: cols 0-63 = V_h0 dims, col 64 = ones
            # vch1[c]: [128 keys, 128]: col 0 = ones, cols 1-63 = 0,
            #          cols 64-127 = V_h1 dims
            vch0 = [
                persist.tile([128, 65], bf16, name=f"vch0_{c}", tag=f"vch0_{c}")
                for c in range(NKC)
            ]
            vch1 = [
                persist.tile([128, 128], bf16, name=f"vch1_{c}", tag=f"vch1_{c}")
                for c in range(NKC)
            ]
            den_sb = persist.tile([66, n], f32, name="den", tag="den")
            for c in range(NKC):
                nc.vector.memset(vch0[c][:, 64:65], 1.0)
                nc.vector.memset(vch1[c][:, 0:1], 1.0)
                nc.vector.memset(vch1[c][:, 1:64], 0.0)

            def dma_xtiles(t, nk2):
                xtiles = []
                for dc in range(4):
                    xtile = xtp.tile(
                        [128, 1024], bf16, name=f"x_{t}{nk2}{dc}", tag="xt"
                    )
                    nc.sync.dma_start(out=xtile, in_=xt[t][dc, nk2])
                    xtiles.append(xtile)
                return xtiles

            def qk_store(t, nk, ppsum_or_pair):
                """Write projected q/k into the duplicated per-head layout."""
                dst = qtd if t == "q" else ktd
                for h in range(2):
                    hh = slice(h * 64, (h + 1) * 64)
                    other = slice(64, 128) if h == 0 else slice(0, 64)
                    if isinstance(ppsum_or_pair, tuple):
                        pp_lo, pp_hi_sb = ppsum_or_pair
                        if zero_bias:
                            nc.vector.tensor_add(
                                out=dst[h][nk][hh, :],
                                in0=pp_lo[hh, :],
                                in1=pp_hi_sb[hh, :],
                            )
                        else:
                            nc.vector.scalar_tensor_tensor(
                                out=dst[h][nk][hh, :],
                                in0=pp_lo[hh, :],
                                scalar=bsb[t][hh, :],
                                in1=pp_hi_sb[hh, :],
                                op0=mybir.AluOpType.add,
                                op1=mybir.AluOpType.add,
                            )
                    else:
                        ppsum = ppsum_or_pair
                        if zero_bias:
                            nc.vector.tensor_copy(
                                out=dst[h][nk][hh, :], in_=ppsum[hh, :]
                            )
                        else:
                            nc.vector.tensor_scalar_add(
                                out=dst[h][nk][hh, :],
                                in0=ppsum[hh, :],
                                scalar1=bsb[t][hh, :],
                            )
                    nc.sync.dma_start(
                        out=dst[h][nk][other, :], in_=dst[h][nk][hh, :]
                    )

            # ---- phase 1 (full 128x128 mode): k and v projections for all
            # chunks, q for nk2=0 only.  The rest of q is projected inside
            # phase 2 in the tiled mode (split contraction).
            for nk2 in range(n // 1024):
                for t in ("k", "v", "q"):
                    if t == "q" and nk2 > 0:
                        continue
                    xtiles = dma_xtiles(t, nk2)
                    for half in range(2):
                        nk = nk2 * 2 + half
                        hs = slice(half * 512, (half + 1) * 512)
                        ppsum = psA.tile([128, 512], f32, name=f"pp_{t}{nk}", tag="s")
                        for dc in range(4):
                            nc.tensor.matmul(
                                ppsum,
                                wsb[t][:, dc, :],
                                xtiles[dc][:, hs],
                                start=(dc == 0),
                                stop=(dc == 3),
                            )
                        if t == "v":
                            if zero_bias:
                                nc.vector.tensor_copy(out=vt_t[nk], in_=ppsum)
                            else:
                                nc.vector.tensor_scalar_add(
                                    out=vt_t[nk], in0=ppsum, scalar1=bsb[t]
                                )
                        else:
                            qk_store(t, nk, ppsum)
                    if t == "v":
                        # v^T chunks -> per-head augmented layouts
                        for c in range(nk2 * 8, nk2 * 8 + 8):
                            pt = psA.tile([128, 512], bf16, name=f"pt{c}", tag="s")
                            nc.tensor.transpose(
                                pt[:, 0:128],
                                vt_t[c // 4][:, (c % 4) * 128 : (c % 4 + 1) * 128],
                                ident,
                            )
                            nc.vector.tensor_copy(
                                out=vch0[c][:, 0:64], in_=pt[:, 0:64]
                            )
                            nc.vector.tensor_copy(
                                out=vch1[c][:, 64:128], in_=pt[:, 64:128]
                            )

            def emit_qproj_tiled(nk2):
                """Project q chunk pair nk2 in the 64x128 tiled mode:
                split the 128-deep contraction across the two PE tiles."""
                xtiles = dma_xtiles("q", nk2)
                for half in range(2):
                    nk = nk2 * 2 + half
                    hs = slice(half * 512, (half + 1) * 512)
                    pp_lo = psB.tile([128, 512], f32, name=f"ql_{nk}", tag="oy")
                    pp_hi = psB.tile([128, 512], f32, name=f"qh_{nk}", tag="oy")
                    for dc in range(4):
                        nc.tensor.matmul(
                            pp_lo,
                            wsb["q"][0:64, dc, :],
                            xtiles[dc][0:64, hs],
                            start=(dc == 0),
                            stop=(dc == 3),
                            skip_group_check=True,
                        )
                        nc.tensor.matmul(
                            pp_hi,
                            wsb["q"][64:128, dc, :],
                            xtiles[dc][64:128, hs],
                            start=(dc == 0),
                            stop=(dc == 3),
                            skip_group_check=True,
                        )
                    pp_hi_sb = ohp.tile([128, 512], f32, name=f"qhs_{nk}", tag="oh")
                    nc.vector.tensor_copy(out=pp_hi_sb, in_=pp_hi)
                    qk_store("q", nk, (pp_lo, pp_hi_sb))

            def emit_outproj(qc):
                ot_prev = ot_tiles[qc]
                qs = slice(qc * 512, (qc + 1) * 512)
                for mt in range(4):
                    for h in range(2):
                        hp = slice(h * 64, (h + 1) * 64)
                        y_ps = psB.tile(
                            [128, 512], f32, name=f"y_{h}_{qc}_{mt}", tag="oy"
                        )
                        nc.tensor.matmul(
                            y_ps,
                            wosb[hp, mt, :],
                            ot_prev[hp, :],
                            start=True,
                            stop=True,
                            skip_group_check=True,
                        )
                        y_sb = ysbp.tile(
                            [128, 512], bf16, name=f"ysb_{h}_{qc}_{mt}", tag="ysb"
                        )
                        nc.vector.tensor_copy(out=y_sb, in_=y_ps)
                        nc.sync.dma_start(out=y_out[h][mt, qc], in_=y_sb)

            # ---- phase 2+3: attention + out-projection + deferred q
            # projections, all in 64x128 row-tiled mode, one head at a time.
            # s/exp run one block ahead of PV so the PE never waits on exp.
            ot_tiles = {}
            for qc in range(NQC):
                qs = slice(qc * 512, (qc + 1) * 512)
                ot_stack = otp.tile([128, 512], bf16, name=f"ot{qc}", tag="ot")
                ot_tiles[qc] = ot_stack
                for h in range(2):
                    vch = vch0 if h == 0 else vch1
                    ow = 65 if h == 0 else 128
                    o_lo = o_hi = None
                    pend = None

                    def emit_pv(blk):
                        nonlocal o_lo, o_hi
                        if o_lo is None:
                            o_lo = psB.tile(
                                [128, 512], f32, name=f"olo_{h}_{qc}", tag="oy"
                            )
                            o_hi = psB.tile(
                                [128, 512], f32, name=f"ohi_{h}_{qc}", tag="oy"
                            )
                        k0, blen, e_sb = blk
                        for j in range(blen):
                            kc = k0 + j
                            st, sp = (kc == 0), (kc == NKC - 1)
                            es = e_sb[:, j * 512 : (j + 1) * 512]
                            nc.tensor.matmul(
                                o_lo[0:ow, :],
                                vch[kc][0:64, :],
                                es[0:64, :],
                                start=st,
                                stop=sp,
                                skip_group_check=True,
                            )
                            nc.tensor.matmul(
                                o_hi[0:ow, :],
                                vch[kc][64:128, :],
                                es[64:128, :],
                                start=st,
                                stop=sp,
                                skip_group_check=True,
                            )

                    for bi, (k0, blen) in enumerate(blocks):
                        s_ps = psA.tile(
                            [128, blen * 512], f32, name=f"s_{h}_{qc}_{k0}", tag="s"
                        )
                        for j in range(blen):
                            kc = k0 + j
                            hp = slice(0, 64) if kc % 2 == 0 else slice(64, 128)
                            nc.tensor.matmul(
                                s_ps[:, j * 512 : (j + 1) * 512],
                                ktd[h][kc // 4][hp, (kc % 4) * 128 : (kc % 4 + 1) * 128],
                                qtd[h][qc][hp, :],
                                start=True,
                                stop=True,
                                skip_group_check=True,
                            )
                        e_sb = ep.tile(
                            [128, blen * 512], bf16, name=f"e_{h}_{qc}_{k0}", tag="e"
                        )
                        nc.scalar.activation(e_sb, s_ps, Exp, scale=0.125)
                        if bi == 1:
                            # slot deferred work here, before the first PV
                            # allocates the o PSUM banks
                            if h == 0 and qc > 0:
                                emit_outproj(qc - 1)
                            if h == 1 and qc in (0, 2, 4):
                                emit_qproj_tiled(qc // 2 + 1)
                        if pend is not None:
                            emit_pv(pend)
                        pend = (k0, blen, e_sb)
                    emit_pv(pend)

                    # DVE may read only one PSUM input: stage o_hi in SBUF
                    o_hi_sb = ohp.tile([128, 512], f32, name=f"ohs_{h}_{qc}", tag="oh")
                    nc.vector.tensor_copy(
                        out=o_hi_sb[0:ow, :], in_=o_hi[0:ow, :]
                    )
                    if h == 0:
                        nc.vector.tensor_add(
                            out=ot_stack[0:64, :],
                            in0=o_lo[0:64, :],
                            in1=o_hi_sb[0:64, :],
                        )
                        nc.vector.tensor_add(
                            out=den_sb[64:65, qs],
                            in0=o_lo[64:65, :],
                            in1=o_hi_sb[64:65, :],
                        )
                    else:
                        nc.vector.tensor_add(
                            out=ot_stack[64:128, :],
                            in0=o_lo[64:128, :],
                            in1=o_hi_sb[64:128, :],
                        )
                        nc.vector.tensor_add(
                            out=den_sb[0:1, qs],
                            in0=o_lo[0:1, :],
                            in1=o_hi_sb[0:1, :],
                        )
            emit_outproj(NQC - 1)
            nc.sync.dma_start(out=den_out[0:1, :], in_=den_sb[64:65, :])
            nc.sync.dma_start(out=den_out[1:2, :], in_=den_sb[0:1, :])
    nc.finalize()
    return nc


def get_nc(n=_N, zero_bias=False):
    key = (n, zero_bias)
    if key not in _nc_cache:
        _nc_cache[key] = _build(n, zero_bias)
    return _nc_cache[key]


def make_in_maps(Q, K, V, Wq, bq, Wk, bk, Wv, bv, Wo, bo, n=_N):
    """Per-core input dicts (host-side sharding / layout prep)."""
    bf = np.float16

    def tile_xt(X):
        # [n, 512] -> X^T tiled as [4, n//1024, 128, 1024]
        xt = X[:n].T.astype(bf)  # [512, n]
        return np.ascontiguousarray(
            xt.reshape(4, 128, n // 1024, 1024).transpose(0, 2, 1, 3)
        )

    xts = {}
    for b in range(_B):
        xts[b] = {
            "xtq": tile_xt(Q[b]),
            "xtk": tile_xt(K[b]),
            "xtv": tile_xt(V[b]),
        }
    in_maps = []
    for c in range(_NCORES):
        b, p = divmod(c, 4)
        off = p * 128
        m = dict(xts[b])
        for t, W, bias in (("q", Wq, bq), ("k", Wk, bk), ("v", Wv, bv)):
            m[f"w{t}"] = np.ascontiguousarray(
                W[:, off : off + 128].reshape(4, 128, 128).transpose(1, 0, 2).astype(bf)
            )
            m[f"b{t}"] = np.ascontiguousarray(bias[off : off + 128].reshape(128, 1))
        m["wo"] = np.ascontiguousarray(
            Wo[off : off + 128].reshape(128, 4, 128).astype(bf)
        )
        in_maps.append(m)
    return in_maps


def assemble(results, bo, n=_N):
    """Cross-core reduction: normalize by softmax denominators, sum heads,
    add output bias, restore [B, N, D] layout."""
    out = np.empty((_B, n, _D), np.float32)
    for b in range(_B):
        acc = np.zeros((_D, n), np.float32)
        for p in range(4):
            r = results[4 * b + p]
            for h in range(2):
                y = r[f"y{h}"].transpose(0, 2, 1, 3).reshape(_D, n)
                acc += y.astype(np.float32) / r["den"][h][None, :]
        out[b] = acc.T + bo
    return out


def kernel(Q, K, V, Wq, bq, Wk, bk, Wv, bv, Wo, bo):
    from concourse import bass_utils

    args = [np.asarray(a, np.float32) for a in (Q, K, V, Wq, bq, Wk, bk, Wv, bv, Wo, bo)]
    Q, K, V, Wq, bq, Wk, bk, Wv, bv, Wo, bo = args
    zb = not (np.any(bq) or np.any(bk) or np.any(bv))
    nc = get_nc(zero_bias=zb)
    in_maps = make_in_maps(Q, K, V, Wq, bq, Wk, bk, Wv, bv, Wo, bo)
    res = bass_utils.run_bass_kernel_spmd(
        nc, in_maps, core_ids=list(range(_NCORES))
    )
    return assemble(res.results, bo)
